# revision 1
# baseline (speedup 1.0000x reference)
"""Chamfer loss (B=4, N=M=8192, D=3) on 8 NeuronCores via Bass/Tile.

Strategy:
  - Shard: 8 cores = 4 batches x 2 halves of the gts (i) axis.
    Each core computes P[i, j] = ||gts_i - preds_j||^2 for its 4096 i's
    against all 8192 j's, flash-style (P never materialized in DRAM).
  - P tiles come straight out of one matmul via the augmented contraction
      P[i,j] = sum_k A[i,k] * B[j,k],  A = [-2*gts, 1, ||gts||^2],
                                       B = [preds, ||preds||^2, 1]
    with each factor split hi/lo into fp16 (K = 2*2*5 = 20) so the PE runs
    at full (1 cycle/row) rate while keeping ~2^-22 relative accuracy.
  - ScalarE casts each PSUM tile to fp16 in SBUF; VectorE (the bottleneck)
    runs fp16 2x-mode running minima: an elementwise-min column accumulator
    [128, 8192] and a wide row accumulator W reduced once per i-tile.
  - Column finale: transpose the column accumulator through the PE against
    an identity matrix, then 3D-AP min-reduce -> colmin partials [128, 64].
  - Host: sum row minima; elementwise-min colmin partials across the two
    i-half cores of each batch, then sum.
"""

import numpy as np
from contextlib import ExitStack

from concourse import bass, bacc, mybir
from concourse import tile
from concourse.bass_utils import run_bass_kernel_spmd

B, N, M, D = 4, 8192, 8192, 3
N_CORES = 8
N_I = N // 2          # i rows per core
CH = 2048             # j chunk width (4 PSUM banks)
KDIM = 20             # augmented contraction after fp16 hi/lo split
F16 = mybir.dt.float16
F32 = mybir.dt.float32
BIG = 60000.0         # > max possible P (~120), < fp16 max


def build_nc(n_i=N_I, m=M, ch=CH, trace_sim=False, repeat=1,
             do_row=True, do_col=True, do_act=True,
             wide_act=True, halve_reduce=True, t0_direct=True, gps_col=False,
             big_s=False, ttr_row=False, deep_bufs=False, packed_pe=False,
             n1024=False):
    """Build the per-core Bass program (same NEFF on all 8 cores).

    repeat: emit the main loop `repeat` times (min is idempotent, results
    unchanged) — used to measure marginal HW exec time per pass.
    do_row/do_col/do_act: ablation switches (wrong results when False).
    """
    NT = n_i // 128       # i-tiles
    NCH = m // ch         # chunks per i-tile
    NMM = ch // 512       # matmuls per chunk
    NBLK = m // 128       # col-finale transpose blocks
    GRP = min(16, NBLK)   # blocks per finale group ([128, GRP*128] f32 PSUM)
    NGRP = NBLK // GRP
    assert n_i % 128 == 0 and m % ch == 0 and ch % 512 == 0 and NBLK % GRP == 0

    nc = bacc.Bacc("TRN2", target_bir_lowering=False, debug=False)
    kp = 128 if packed_pe else KDIM
    lhsT_d = nc.dram_tensor("lhsT", [kp, n_i], F16, kind="ExternalInput").ap()
    rhs_d = nc.dram_tensor("rhs", [kp, m], F16, kind="ExternalInput").ap()
    ident_d = nc.dram_tensor("ident", [128, 128], F16, kind="ExternalInput").ap()
    rowmin_d = nc.dram_tensor("rowmin", [128, NT], F32, kind="ExternalOutput").ap()
    colmin_d = nc.dram_tensor("colmin", [128, NBLK], F32, kind="ExternalOutput").ap()

    mn = mybir.AluOpType.min

    with tile.TileContext(nc, trace_sim=trace_sim) as tc, ExitStack() as ctx:
        singles = ctx.enter_context(tc.tile_pool(name="singles", bufs=1))
        spool = ctx.enter_context(tc.tile_pool(name="spool", bufs=3))
        wpool = ctx.enter_context(tc.tile_pool(name="wpool", bufs=2))
        pspool = ctx.enter_context(tc.tile_pool(name="pspool", bufs=2, space="PSUM"))

        lhsT_sb = singles.tile([kp, n_i], F16)
        nc.sync.dma_start(out=lhsT_sb, in_=lhsT_d)
        rhs_sb = singles.tile([kp, m], F16)
        nc.sync.dma_start(out=rhs_sb, in_=rhs_d)
        ident_sb = singles.tile([128, 128], F16)
        nc.sync.dma_start(out=ident_sb, in_=ident_d)

        colacc = singles.tile([128, m], F16)
        if not t0_direct:
            nc.gpsimd.memset(colacc, BIG)
        rowacc = singles.tile([128, NT], F32)
        colfin = singles.tile([128, NBLK], F32)
        if not do_row:
            nc.vector.memset(rowacc, 0.0)

        if ttr_row:
            # Row path via tensor_tensor_reduce over chunk PAIRS of one
            # i-tile: out = min(S_even, S_odd) (discarded scratch) while the
            # fused min-reduce with chained initial value lands the row min
            # directly in rowacc — the whole reduce chain disappears.
            assert NCH % 2 == 0
            rowacc_b = singles.tile([128, NT], F32)
            if NCH < 4:
                nc.vector.memset(rowacc_b, BIG)
            first_pass = True
            for t in [tt for _ in range(repeat) for tt in range(NT)]:
                t0d = t0_direct and t == 0 and first_pass
                first_pass = False
                schunks = []
                for c in range(NCH):
                    if t0d:
                        s_cur = colacc[:, ch * c:ch * (c + 1)]
                    else:
                        s = spool.tile([128, ch], F16, tag="s", bufs=5)
                        s_cur = s
                    ps = pspool.tile([128, ch], F32, tag="ps", bufs=2)
                    for q in range(NMM):
                        nc.tensor.matmul(
                            ps[:, 512 * q:512 * (q + 1)],
                            lhsT_sb[:, 128 * t:128 * (t + 1)],
                            rhs_sb[:, ch * c + 512 * q: ch * c + 512 * (q + 1)],
                            start=True, stop=True,
                        )
                    nc.scalar.copy(out=s_cur, in_=ps)
                    schunks.append(s_cur)
                    if not t0d:
                        cs = colacc[:, ch * c:ch * (c + 1)]
                        nc.vector.tensor_tensor(out=cs, in0=cs, in1=s_cur, op=mn)
                    if c % 2 == 1:
                        wsc = wpool.tile([128, ch], F16, tag="w")
                        acc = rowacc if c == 1 else rowacc_b
                        nc.vector.tensor_tensor_reduce(
                            out=wsc, in0=schunks[c - 1], in1=s_cur, scale=1.0,
                            scalar=BIG, op0=mn, op1=mn,
                            accum_out=acc[:, t:t + 1],
                        )
            # combine the two per-i-tile partial row minima (tiny)
            nc.vector.tensor_tensor(out=rowacc, in0=rowacc, in1=rowacc_b, op=mn)

        elif big_s:
            # fused variant: one [128, m] S buffer per i-tile -> single wide
            # col TT + pure log-halving row chain (fewer DVE instructions)
            first_pass = True
            for t in [tt for _ in range(repeat) for tt in range(NT)]:
                t0d = t0_direct and t == 0 and first_pass
                first_pass = False
                if t0d:
                    sfull = colacc
                else:
                    sfull = spool.tile([128, m], F16, tag="sbig", bufs=2)
                for c in range(NCH):
                    ps = pspool.tile([128, ch], F32, tag="ps", bufs=2)
                    for q in range(NMM):
                        nc.tensor.matmul(
                            ps[:, 512 * q:512 * (q + 1)],
                            lhsT_sb[:, 128 * t:128 * (t + 1)],
                            rhs_sb[:, ch * c + 512 * q: ch * c + 512 * (q + 1)],
                            start=True, stop=True,
                        )
                    nc.scalar.copy(out=sfull[:, ch * c:ch * (c + 1)], in_=ps)
                if not t0d:
                    nc.vector.tensor_tensor(
                        out=colacc, in0=colacc, in1=sfull, op=mn)
                cur, width, hidx = sfull, m, 0
                while width > 512:
                    nxt = wpool.tile([128, width // 2], F16, tag=f"h{hidx}")
                    nc.vector.tensor_tensor(
                        out=nxt, in0=cur[:, :width // 2],
                        in1=cur[:, width // 2:], op=mn)
                    cur, width, hidx = nxt, width // 2, hidx + 1
                nc.vector.tensor_reduce(
                    out=rowacc[:, t:t + 1], in_=cur,
                    axis=mybir.AxisListType.X, op=mn)

        first_pass = True
        for t in ([] if (big_s or ttr_row) else
                  [tt for _ in range(repeat) for tt in range(NT)]):
            t0d = t0_direct and t == 0 and first_pass
            w = wpool.tile([128, ch], F16, tag="w",
                           bufs=3 if deep_bufs else None)
            for c in range(NCH):
                if t0d:
                    # i-tile 0: ScalarE writes colacc directly; no col TT needed
                    s_cur = colacc[:, ch * c:ch * (c + 1)]
                elif c == 0:
                    s_cur = w  # first chunk: ScalarE writes the row accum directly
                else:
                    s = spool.tile([128, ch], F16, tag="s",
                                   bufs=5 if deep_bufs else None)
                    s_cur = s
                if wide_act and n1024:
                    # halve PE instruction count: fp16 moving operand max is
                    # 1024, each MM spans 2 PSUM banks
                    ps = pspool.tile([128, ch], F32, tag="ps", bufs=2)
                    for q in range(ch // 1024):
                        nc.tensor.matmul(
                            ps[:, 1024 * q:1024 * (q + 1)],
                            lhsT_sb[:, 128 * t:128 * (t + 1)],
                            rhs_sb[:, ch * c + 1024 * q: ch * c + 1024 * (q + 1)],
                            start=True, stop=True,
                        )
                    if do_act:
                        nc.scalar.copy(out=s_cur, in_=ps)
                elif wide_act:
                    ps = pspool.tile([128, ch], F32, tag="ps", bufs=2)
                    for q in range(NMM):
                        if packed_pe:
                            bp = 32 * (q % 4)
                            nc.tensor.matmul(
                                ps[:, 512 * q:512 * (q + 1)],
                                lhsT_sb[bp:bp + KDIM, 128 * t:128 * (t + 1)],
                                rhs_sb[bp:bp + KDIM,
                                       ch * c + 512 * q: ch * c + 512 * (q + 1)],
                                start=True, stop=True,
                                tile_position=(bp, 0),
                            )
                        else:
                            nc.tensor.matmul(
                                ps[:, 512 * q:512 * (q + 1)],
                                lhsT_sb[:, 128 * t:128 * (t + 1)],
                                rhs_sb[:, ch * c + 512 * q: ch * c + 512 * (q + 1)],
                                start=True, stop=True,
                            )
                    if do_act:
                        nc.scalar.copy(out=s_cur, in_=ps)
                else:
                    for q in range(NMM):
                        ps = pspool.tile([128, 512], F32, tag="ps", bufs=4)
                        nc.tensor.matmul(
                            ps,
                            lhsT_sb[:, 128 * t:128 * (t + 1)],
                            rhs_sb[:, ch * c + 512 * q: ch * c + 512 * (q + 1)],
                            start=True, stop=True,
                        )
                        if do_act:
                            nc.scalar.copy(out=s_cur[:, 512 * q:512 * (q + 1)], in_=ps)
                if t0d and do_row:
                    # row accum for i-tile 0 reads the colacc slices
                    if c == 0:
                        pass  # w seeded at c == 1 from colacc chunk 0
                    elif c == 1:
                        nc.vector.tensor_tensor(
                            out=w, in0=colacc[:, 0:ch], in1=s_cur, op=mn)
                    else:
                        nc.vector.tensor_tensor(out=w, in0=w, in1=s_cur, op=mn)
                elif do_row and c > 0:
                    nc.vector.tensor_tensor(out=w, in0=w, in1=s_cur, op=mn)
                if do_col and not t0d:
                    # column path: running elementwise min
                    cs = colacc[:, ch * c:ch * (c + 1)]
                    eng = nc.gpsimd if (gps_col and c == NCH - 1) else nc.vector
                    eng.tensor_tensor(out=cs, in0=cs, in1=s_cur, op=mn)
            first_pass = False
            if do_row:
                if halve_reduce and ch >= 2048:
                    h1 = wpool.tile([128, ch // 2], F16, tag="h1")
                    nc.vector.tensor_tensor(
                        out=h1, in0=w[:, :ch // 2], in1=w[:, ch // 2:], op=mn)
                    h2 = wpool.tile([128, ch // 4], F16, tag="h2")
                    nc.vector.tensor_tensor(
                        out=h2, in0=h1[:, :ch // 4], in1=h1[:, ch // 4:], op=mn)
                    nc.vector.tensor_reduce(
                        out=rowacc[:, t:t + 1], in_=h2,
                        axis=mybir.AxisListType.X, op=mn)
                else:
                    nc.vector.tensor_reduce(
                        out=rowacc[:, t:t + 1], in_=w,
                        axis=mybir.AxisListType.X, op=mn)

        # column finale: partition-axis min via PE transpose + free-axis reduce
        for g in range(NGRP):
            if wide_act and GRP * 128 == ch:
                pst = pspool.tile([128, GRP * 128], F32, tag="ps", bufs=2)
            else:
                pst = pspool.tile([128, GRP * 128], F32, tag="psfin", bufs=1)
            for k in range(GRP):
                blk = g * GRP + k
                nc.tensor.matmul(
                    pst[:, 128 * k:128 * (k + 1)],
                    colacc[:, 128 * blk:128 * (blk + 1)],
                    ident_sb,
                    start=True, stop=True,
                )
            nc.vector.tensor_reduce(
                out=colfin[:, g * GRP:(g + 1) * GRP],
                in_=pst.rearrange("p (b x) -> p b x", x=128),
                axis=mybir.AxisListType.X, op=mn,
            )

        nc.sync.dma_start(out=rowmin_d, in_=rowacc)
        nc.sync.dma_start(out=colmin_d, in_=colfin)
    nc.compile()
    return nc


def _split16(x):
    hi = x.astype(np.float16)
    lo = (x - hi.astype(np.float32)).astype(np.float16)
    return hi, lo


def prep_core_inputs(gts_b, preds_b):
    """Augmented, fp16 hi/lo split operands for one core.

    gts_b: [n_i, 3] f32 (this core's i rows), preds_b: [m, 3] f32.
    Returns lhsT [20, n_i] f16, rhs [20, m] f16.
    """
    gts_b = np.asarray(gts_b, dtype=np.float32)
    preds_b = np.asarray(preds_b, dtype=np.float32)
    xx = np.sum(gts_b * gts_b, axis=1, dtype=np.float32)
    yy = np.sum(preds_b * preds_b, axis=1, dtype=np.float32)
    ones_a = np.ones((gts_b.shape[0],), np.float32)
    ones_b = np.ones((preds_b.shape[0],), np.float32)
    A = np.concatenate([-2.0 * gts_b, ones_a[:, None], xx[:, None]], axis=1)  # [n,5]
    Bm = np.concatenate([preds_b, yy[:, None], ones_b[:, None]], axis=1)      # [m,5]
    Ah, Al = _split16(A)
    Bh, Bl = _split16(Bm)
    lhsT = np.concatenate([Ah, Ah, Al, Al], axis=1).T.copy()  # [20, n]
    rhs = np.concatenate([Bh, Bl, Bh, Bl], axis=1).T.copy()   # [20, m]
    return lhsT, rhs


def prep_core_inputs_packed(gts_b, preds_b):
    """prep_core_inputs replicated at partition offsets 0/32/64/96 for
    tile_position row-group packing."""
    lhsT, rhs = prep_core_inputs(gts_b, preds_b)
    Lp = np.zeros((128, lhsT.shape[1]), np.float16)
    Rp = np.zeros((128, rhs.shape[1]), np.float16)
    for g in range(4):
        Lp[32 * g:32 * g + KDIM] = lhsT
        Rp[32 * g:32 * g + KDIM] = rhs
    return Lp, Rp


def combine_outputs(results, m=M):
    """results: list of 8 dicts with 'rowmin' [128, NT] and 'colmin' [128, NBLK]."""
    total = 0.0
    for b in range(len(results) // 2):
        r0, r1 = results[2 * b], results[2 * b + 1]
        total += np.sum(r0["rowmin"], dtype=np.float64)
        total += np.sum(r1["rowmin"], dtype=np.float64)
        c0 = r0["colmin"].T.reshape(-1)  # colfin[p, blk] -> j = blk*128 + p
        c1 = r1["colmin"].T.reshape(-1)
        total += np.sum(np.minimum(c0, c1), dtype=np.float64)
    return np.float32(total)


_NC_CACHE = {}


def kernel(gts, preds, _trace=False):
    gts = np.asarray(gts, dtype=np.float32)
    preds = np.asarray(preds, dtype=np.float32)
    assert gts.shape == (B, N, D) and preds.shape == (B, M, D)

    key = (N_I, M, CH)
    if key not in _NC_CACHE:
        _NC_CACHE[key] = build_nc()
    nc = _NC_CACHE[key]

    ident = np.eye(128, dtype=np.float16)
    in_maps = []
    for c in range(N_CORES):
        b, half = c // 2, c % 2
        lhsT, rhs = prep_core_inputs(
            gts[b, half * N_I:(half + 1) * N_I], preds[b]
        )
        in_maps.append({"lhsT": lhsT, "rhs": rhs, "ident": ident})

    res = run_bass_kernel_spmd(nc, in_maps, list(range(N_CORES)), trace=_trace)
    out = combine_outputs(res.results)
    if _trace:
        return out, res
    return out



# revision 2
# speedup vs baseline: 3.5745x; 3.5745x over previous
"""Chamfer loss (B=4, N=M=8192, D=3) on 8 NeuronCores — windowed-kNN Bass kernel.

Key idea: both point sets are sorted by norm (host-side permutation; the loss
is permutation invariant). For a tile of 128 norm-sorted gts rows, the true
nearest pred lies within a +-512 window of the matching pred norm-rank for all
but a handful of points. A rigorous certificate (host-side, cheap) identifies
every row whose window COULD miss its nearest neighbor:
   U_i  = upper bound on nn dist^2 (best of k=256 rank-nearest preds)
   m_i  = min norm-gap from x_i to the window's edge norms
   certified exact iff U_i <= m_i^2   (any pred outside the window is farther
                                       than m_i in norm, hence in distance)
Cert-failing rows (~30-130 of 8192 per batch) are re-done exactly in one extra
full-scan tile per core; cert-failing preds get their column minima from one
extra transposed full-scan tile per core. Everything else: P tiles via the
augmented fp16 hi/lo matmul (as the baseline), ScalarE PSUM->SBUF cast, DVE
running column minima + row halving, GPSIMD final row reduce.

Per-core program (same NEFF on all 8 cores; core = (batch, half), rows of a
half are the even/odd norm-ranks so the window schedule is half-independent):
  - 32 window tiles: 1 matmul [20,128]x[20,1024] -> PSUM, ScalarE cast,
    DVE col TT-min into colacc[:, s(t):s(t)+1024], DVE halve x2 + GPSIMD
    min-reduce -> rowacc[:, t]
  - 1 flagged-rows tile: 128 flagged gts vs all 8192 preds (row path only)
  - 1 transposed tile: 128 flagged preds vs all 8192 gts (row path only ->
    exact column minima for those preds)
  - colacc finale: PE transpose + 3D-AP min-reduce -> colmin [128, 64]
"""

import numpy as np
from contextlib import ExitStack

from concourse import bass, bacc, mybir
from concourse import tile
from concourse.bass_utils import run_bass_kernel_spmd

B, N, M, D = 4, 8192, 8192, 3
N_CORES = 8
N_I = N // 2          # rows per core
NT = N_I // 128       # window tiles per core (32)
W = 1024              # window width
KCERT = 256           # rank-neighbor candidates for the cert upper bound
XCH = 1024            # chunk width for the extra full-scan tiles
NXCH = M // XCH
KDIM = 20
F16 = mybir.dt.float16
F32 = mybir.dt.float32
BIG = 60000.0
NBLK = M // 128       # col-finale transpose blocks
GRP = 8
NGRP = NBLK // GRP


def window_starts():
    """Static window start per tile (multiple of 16, shared by both halves)."""
    out = []
    for t in range(NT):
        c = 256 * t + 128
        s = min(max(c - W // 2, 0), M - W)
        out.append((s // 16) * 16)
    return out


WSTARTS = window_starts()


def build_nc(trace_sim=False, repeat=1):
    nc = bacc.Bacc("TRN2", target_bir_lowering=False, debug=False)
    lhsT_d = nc.dram_tensor("lhsT", [KDIM, N_I], F16, kind="ExternalInput").ap()
    rhsp_d = nc.dram_tensor("rhsp", [KDIM, NT * W], F16, kind="ExternalInput").ap()
    rhsf_d = nc.dram_tensor("rhsf", [KDIM, M], F16, kind="ExternalInput").ap()
    lhsx_d = nc.dram_tensor("lhsx", [KDIM, 128], F16, kind="ExternalInput").ap()
    lhst_d = nc.dram_tensor("lhst", [KDIM, 128], F16, kind="ExternalInput").ap()
    rhsg_d = nc.dram_tensor("rhsg", [KDIM, N], F16, kind="ExternalInput").ap()
    ident_d = nc.dram_tensor("ident", [128, 128], F16, kind="ExternalInput").ap()
    rowmin_d = nc.dram_tensor("rowmin", [128, NT + 2], F32, kind="ExternalOutput").ap()
    colmin_d = nc.dram_tensor("colmin", [128, NBLK], F32, kind="ExternalOutput").ap()

    mn = mybir.AluOpType.min

    with tile.TileContext(nc, trace_sim=trace_sim) as tc, ExitStack() as ctx:
        singles = ctx.enter_context(tc.tile_pool(name="singles", bufs=1))
        spool = ctx.enter_context(tc.tile_pool(name="spool", bufs=4))
        wpool = ctx.enter_context(tc.tile_pool(name="wpool", bufs=3))
        pspool = ctx.enter_context(tc.tile_pool(name="pspool", bufs=3, space="PSUM"))

        lhsT_sb = singles.tile([KDIM, N_I], F16)
        nc.sync.dma_start(out=lhsT_sb, in_=lhsT_d)
        rhsp_sb = singles.tile([KDIM, NT * W], F16)
        nc.sync.dma_start(out=rhsp_sb, in_=rhsp_d)
        rhsf_sb = singles.tile([KDIM, M], F16)
        nc.sync.dma_start(out=rhsf_sb, in_=rhsf_d)
        lhsx_sb = singles.tile([KDIM, 128], F16)
        nc.sync.dma_start(out=lhsx_sb, in_=lhsx_d)
        lhst_sb = singles.tile([KDIM, 128], F16)
        nc.sync.dma_start(out=lhst_sb, in_=lhst_d)
        rhsg_sb = singles.tile([KDIM, N], F16)
        nc.sync.dma_start(out=rhsg_sb, in_=rhsg_d)
        ident_sb = singles.tile([128, 128], F16)
        nc.sync.dma_start(out=ident_sb, in_=ident_d)

        colacc = singles.tile([128, M], F16)
        rowacc = singles.tile([128, NT + 2], F32)
        colfin = singles.tile([128, NBLK], F32)

        for _ in range(repeat):
            nc.vector.memset(colacc, BIG)

            # ---- window tiles ----
            for t in range(NT):
                s = WSTARTS[t]
                ps = pspool.tile([128, W], F32, tag="ps")
                for q in range(W // 512):
                    nc.tensor.matmul(
                        ps[:, 512 * q:512 * (q + 1)],
                        lhsT_sb[:, 128 * t:128 * (t + 1)],
                        rhsp_sb[:, W * t + 512 * q:W * t + 512 * (q + 1)],
                        start=True, stop=True,
                    )
                sf = spool.tile([128, W], F16, tag="s")
                nc.scalar.copy(out=sf, in_=ps)
                # col path
                cs = colacc[:, s:s + W]
                nc.vector.tensor_tensor(out=cs, in0=cs, in1=sf, op=mn)
                # row path: halve chain + reduce on DVE
                h1 = wpool.tile([128, W // 2], F16, tag="h1")
                nc.vector.tensor_tensor(
                    out=h1, in0=sf[:, :W // 2], in1=sf[:, W // 2:], op=mn)
                h2 = wpool.tile([128, W // 4], F16, tag="h2")
                nc.vector.tensor_tensor(
                    out=h2, in0=h1[:, :W // 4], in1=h1[:, W // 4:], op=mn)
                nc.vector.tensor_reduce(
                    out=rowacc[:, t:t + 1], in_=h2,
                    axis=mybir.AxisListType.X, op=mn)

            # ---- extra full-scan tiles (row path only) ----
            for xt, (lhs_one, rhs_all) in enumerate(
                    [(lhsx_sb, rhsf_sb), (lhst_sb, rhsg_sb)]):
                wx = wpool.tile([128, XCH], F16, tag="wx")
                for c in range(NXCH):
                    ps = pspool.tile([128, XCH], F32, tag="ps")
                    for q in range(XCH // 512):
                        nc.tensor.matmul(
                            ps[:, 512 * q:512 * (q + 1)],
                            lhs_one,
                            rhs_all[:, XCH * c + 512 * q:XCH * c + 512 * (q + 1)],
                            start=True, stop=True,
                        )
                    if c == 0:
                        nc.scalar.copy(out=wx, in_=ps)
                    else:
                        sx = spool.tile([128, XCH], F16, tag="sx", bufs=2)
                        nc.scalar.copy(out=sx, in_=ps)
                        nc.vector.tensor_tensor(out=wx, in0=wx, in1=sx, op=mn)
                hx1 = wpool.tile([128, XCH // 2], F16, tag="hx1")
                nc.vector.tensor_tensor(
                    out=hx1, in0=wx[:, :XCH // 2], in1=wx[:, XCH // 2:], op=mn)
                hx2 = wpool.tile([128, XCH // 4], F16, tag="hx2")
                nc.vector.tensor_tensor(
                    out=hx2, in0=hx1[:, :XCH // 4], in1=hx1[:, XCH // 4:], op=mn)
                nc.vector.tensor_reduce(
                    out=rowacc[:, NT + xt:NT + xt + 1], in_=hx2,
                    axis=mybir.AxisListType.X, op=mn)

            # ---- column finale: PE transpose + 3D-AP min reduce ----
            for g in range(NGRP):
                pst = pspool.tile([128, GRP * 128], F32, tag="ps")
                for k in range(GRP):
                    blk = g * GRP + k
                    nc.tensor.matmul(
                        pst[:, 128 * k:128 * (k + 1)],
                        colacc[:, 128 * blk:128 * (blk + 1)],
                        ident_sb,
                        start=True, stop=True,
                    )
                nc.vector.tensor_reduce(
                    out=colfin[:, g * GRP:(g + 1) * GRP],
                    in_=pst.rearrange("p (b x) -> p b x", x=128),
                    axis=mybir.AxisListType.X, op=mn,
                )

        nc.sync.dma_start(out=rowmin_d, in_=rowacc)
        nc.sync.dma_start(out=colmin_d, in_=colfin)
    nc.compile()
    return nc


# ---------------- host-side prep ----------------

def _split16(x):
    hi = x.astype(np.float16)
    lo = (x - hi.astype(np.float32)).astype(np.float16)
    return hi, lo


def _aug_A(pts):
    """gts-side augmentation: P = A . B with A = [-2x, 1, xx]."""
    xx = np.sum(pts * pts, axis=1, dtype=np.float32)
    ones = np.ones((pts.shape[0],), np.float32)
    return np.concatenate([-2.0 * pts, ones[:, None], xx[:, None]], axis=1)


def _aug_B(pts):
    """preds-side augmentation: B = [y, yy, 1]."""
    yy = np.sum(pts * pts, axis=1, dtype=np.float32)
    ones = np.ones((pts.shape[0],), np.float32)
    return np.concatenate([pts, yy[:, None], ones[:, None]], axis=1)


def _pack16(aug):
    """[n, 5] f32 -> [20, n] f16 hi/lo (A-operand layout)."""
    h, l = _split16(aug)
    return np.concatenate([h, h, l, l], axis=1).T.copy()


def _pack16_B(aug):
    h, l = _split16(aug)
    return np.concatenate([h, l, h, l], axis=1).T.copy()


def cert_flags(xs, ys, starts, k=KCERT):
    """Flags over sorted rows xs whose device window might miss the nn.

    Device tile t (of either half-core) covers global sorted ranks
    [256t, 256t+256) and scans preds [starts[t], starts[t]+W).
    """
    n, m = len(xs), len(ys)
    nxs = np.sqrt((xs * xs).sum(1))
    nys = np.sqrt((ys * ys).sum(1))
    flags = np.zeros(n, bool)
    gap = np.zeros(n, np.float32)
    xx = (xs * xs).sum(1)
    yy = (ys * ys).sum(1)
    for t in range(n // 256):
        s = starts[t]
        blk = slice(256 * t, 256 * (t + 1))
        xt = xs[blk]
        c0 = min(max((256 * t + 128) - k // 2, 0), m - k)
        yc = ys[c0:c0 + k]
        Pc = xx[blk][:, None] + yy[c0:c0 + k][None, :] - 2.0 * xt @ yc.T
        U = Pc.min(1)
        nrm = nxs[blk]
        a = np.where(s == 0, np.inf, nrm - nys[s])
        b = np.where(s + W == m, np.inf, nys[s + W - 1] - nrm)
        m2 = np.minimum(a, b) ** 2
        flags[blk] = U > m2 * (1.0 - 1e-3)
        gap[blk] = U - m2
    return flags, gap


def _cap128(idx, gap):
    """Keep at most 128 flagged indices (worst violations first)."""
    if len(idx) <= 128:
        return idx
    order = np.argsort(-gap[idx])
    return np.sort(idx[order[:128]])


_NC_CACHE = {}


def prep_in_maps(gts, preds):
    """Host prep: sort, certify, pack fp16 operands for all 8 cores."""
    ident = np.eye(128, dtype=np.float16)
    starts = WSTARTS
    in_maps = []
    meta = []
    for bb in range(B):
        x, y = gts[bb], preds[bb]
        xord = np.argsort((x * x).sum(1), kind="stable")
        yord = np.argsort((y * y).sum(1), kind="stable")
        xs, ys = x[xord], y[yord]

        fr, gr = cert_flags(xs, ys, starts)
        fc, gc = cert_flags(ys, xs, starts)
        fr_idx = np.where(fr)[0]
        fc_idx = np.where(fc)[0]

        A_s = _aug_A(xs)                      # [N, 5]
        B_s = _aug_B(ys)                      # [M, 5]
        rhsf = _pack16_B(B_s)                 # [20, M] full sorted preds
        rhsg = _pack16_B(_aug_B(xs))          # [20, N] full sorted gts (B-form)
        rhsp = np.concatenate(
            [rhsf[:, starts[t]:starts[t] + W] for t in range(NT)], axis=1)

        for h in range(2):
            rows = np.arange(h, N, 2)         # interleaved half
            lhsT = _pack16(A_s[rows])         # [20, N_I]
            # flagged rows belonging to this half
            fr_h = fr_idx[fr_idx % 2 == h]
            fr_h = _cap128(fr_h, gr)
            lx = np.zeros((128, 5), np.float32)
            lx[:, 3] = 1.0                    # padding rows: P = yy_j >= 0
            if len(fr_h):
                lx[:len(fr_h)] = A_s[fr_h]
            # flagged preds split between the two half-cores
            fc_h = fc_idx[h::2]
            fc_h = _cap128(fc_h, gc)
            lt = np.zeros((128, 5), np.float32)
            lt[:, 3] = 1.0
            if len(fc_h):
                lt[:len(fc_h)] = _aug_A(ys[fc_h])
            in_maps.append({
                "lhsT": lhsT,
                "rhsp": rhsp,
                "rhsf": rhsf,
                "lhsx": _pack16(lx),
                "lhst": _pack16(lt),
                "rhsg": rhsg,
                "ident": ident,
            })
            meta.append({"rows": rows, "fr_h": fr_h, "fc_h": fc_h})
    return in_maps, meta


def kernel(gts, preds, _trace=False):
    gts = np.asarray(gts, dtype=np.float32)
    preds = np.asarray(preds, dtype=np.float32)
    assert gts.shape == (B, N, D) and preds.shape == (B, M, D)

    key = (W, NT)
    if key not in _NC_CACHE:
        _NC_CACHE[key] = build_nc()
    nc = _NC_CACHE[key]

    in_maps, meta = prep_in_maps(gts, preds)
    res = run_bass_kernel_spmd(nc, in_maps, list(range(N_CORES)), trace=_trace)
    out = combine(res.results, meta)
    if _trace:
        return out, res
    return out


def combine(results, meta):
    total = 0.0
    for bb in range(B):
        r0, r1 = results[2 * bb], results[2 * bb + 1]
        m0, m1 = meta[2 * bb], meta[2 * bb + 1]
        # row minima (window tiles), with flagged rows replaced by exact scans
        for r, mt in ((r0, m0), (r1, m1)):
            rowm = r["rowmin"][:, :NT].T.reshape(-1)  # tile t, partition p -> row
            # row index within the half = 128*t + p -> rowm order is [t, p]
            # flagged replacement: local position of flagged row in the half
            fr_h = mt["fr_h"]
            if len(fr_h):
                half_pos = fr_h // 2            # position within this half
                rowm = rowm.copy()
                rowm[half_pos] = r["rowmin"][:len(fr_h), NT]
            total += np.sum(rowm, dtype=np.float64)
        # column minima: min across the 2 half-cores, then flagged overwrite
        c0 = r0["colmin"].T.reshape(-1)
        c1 = r1["colmin"].T.reshape(-1)
        colm = np.minimum(c0, c1)
        for r, mt in ((r0, m0), (r1, m1)):
            fc_h = mt["fc_h"]
            if len(fc_h):
                colm[fc_h] = r["rowmin"][:len(fc_h), NT + 1]
        total += np.sum(colm, dtype=np.float64)
    return np.float32(total)
